# revision 12
# baseline (speedup 1.0000x reference)
"""Trainium2 Bass kernel for nn_MinLoss_12343736009330.

Math: the reference loss is
    loss = sum_{b,s} || pf[b,s] - gf[b,match[b,s]] ||_2
where pf/gf are the per-(batch, source) flattened [L=T*D] signals, and match
is a greedy assignment on the 4x4 Euclidean cdist.  Since
    ||pf[s] - gf[m]||^2 = pn[s] + gn[m] - 2 <pf[s], gf[m]>,
the whole computation reduces to the per-batch 8x8 Gram matrix of the
8 vectors {pf[0..4], gf[0..4]} plus a tiny 4x4 greedy matching.

Sharding: batch axis (16) across 8 cores -> 2 batches/core.  Each core
computes its 2 Gram matrices on the TensorEngine (contraction over t in
128-row tiles; operand columns interleaved (d, v) so that the 8x8 diagonal
blocks of each accumulated matmul hold per-d-slice Gram contributions),
extracts + reduces the diagonal blocks, computes dist = sqrt(pn+gn-2G),
runs the greedy matching on-device with VectorE ops, and writes one loss
scalar per batch.  Host sums the 16 partial scalars.
"""

import os
import sys

import numpy as np

try:
    import concourse.bass as bass  # noqa: F401
except ImportError:
    sys.path.insert(0, "/opt/trn_rl_repo")

import concourse.bass as bass
import concourse.tile as tile
from concourse import bacc, mybir
from concourse.bass_utils import run_bass_kernel_spmd
from concourse.masks import make_identity


def _install_ntff_hook_shim():
    """The bare agent image lacks ``antenv.axon_hooks``, so trace=True under
    axon would ImportError.  Recreate the module with the ctypes-based NTFF
    hook from trn_agent_boot (degrades to hook=None if unavailable)."""
    import types

    try:
        import antenv.axon_hooks  # noqa: F401

        return
    except ImportError:
        pass
    hook = None
    try:
        from trn_agent_boot.trn_boot import _ntff_profile_via_ctypes

        so_path = "/opt/axon/libaxon_pjrt.so"
        if os.path.exists(so_path):
            hook = _ntff_profile_via_ctypes(so_path)
    except Exception:
        hook = None
    import antenv

    mod = types.ModuleType("antenv.axon_hooks")
    mod.get_axon_ntff_profile_hook = lambda: hook  # type: ignore[attr-defined]

    def _set(h):
        nonlocal hook
        hook = h

    mod.set_axon_ntff_profile_hook = _set  # type: ignore[attr-defined]
    sys.modules["antenv.axon_hooks"] = mod
    antenv.axon_hooks = mod


_install_ntff_hook_shim()

F32 = mybir.dt.float32

S, T, B, D = 4, 512, 16, 512
N_CORES = 8
NB = B // N_CORES          # batches per core
NTB = T // 128             # t-blocks per batch
NV = 2 * S                 # 8 vectors per batch (4 preds + 4 gts)

# "bf16": interleave-copy casts to bf16; matmuls at 1 cyc/col (fp32 PSUM).
# "fp32": interleave-copy stays fp32; matmuls at 4 cyc/col (exact).
VARIANT = os.environ.get("MINLOSS_VARIANT", "bf16")
BIG = 1.0e30


def _build_consts() -> np.ndarray:
    """Host-side constant block, DMA'd once: [8, 96] fp32.

    row 0: cols 0:16 iota(16), 16:32 row-id (j//4), 32:48 col-id (j%4),
           48:51 thresholds {4,8,12}, 56:72 the value 99.0.
    rows 0..8, cols 80:88: 8x8 identity (for the flatten matmuls).
    """
    c = np.zeros((8, 96), np.float32)
    j = np.arange(16)
    c[0, 0:16] = j
    c[0, 16:32] = j // 4
    c[0, 32:48] = j % 4
    c[0, 48:51] = [4.0, 8.0, 12.0]
    c[0, 56:72] = 99.0
    c[0:8, 80:88] = np.eye(8, dtype=np.float32)
    return c


CONSTS = _build_consts()


def build_nc(variant: str = VARIANT):
    nc = bacc.Bacc(
        "TRN2",
        target_bir_lowering=False,
        debug=False,
        enable_asserts=True,
        num_devices=N_CORES,
    )
    preds_t = nc.dram_tensor("preds", [S, T, NB, D], F32, kind="ExternalInput").ap()
    gts_t = nc.dram_tensor("gts", [S, NB, T, D], F32, kind="ExternalInput").ap()
    consts_t = nc.dram_tensor("consts", [8, 96], F32, kind="ExternalInput").ap()
    loss_t = nc.dram_tensor("loss", [NB, 1], F32, kind="ExternalOutput").ap()
    gram_t = nc.dram_tensor("gram", [NB, 64], F32, kind="ExternalOutput").ap()

    with tile.TileContext(nc) as tc:
        _build_tile(tc, preds_t, gts_t, consts_t, loss_t, gram_t, variant)

    nc.compile()
    return nc


def _build_tile(tc, preds_t, gts_t, consts_t, loss_t, gram_t, variant):
    nc = tc.nc
    import contextlib

    ctx = contextlib.ExitStack()
    with ctx:
        a_pool = ctx.enter_context(tc.tile_pool(name="a", bufs=4))
        b_pool = ctx.enter_context(tc.tile_pool(name="b", bufs=2 * NTB))
        psum_pool = ctx.enter_context(tc.tile_pool(name="psum", bufs=2, space="PSUM"))
        psumf_pool = ctx.enter_context(tc.tile_pool(name="psumf", bufs=2, space="PSUM"))
        consts_pool = ctx.enter_context(tc.tile_pool(name="consts", bufs=1))
        small_pool = ctx.enter_context(tc.tile_pool(name="small", bufs=2))

        csb = consts_pool.tile([8, 96], F32)
        nc.sync.dma_start(out=csb[:, :], in_=consts_t[:, :])
        ident128 = consts_pool.tile([128, 128], F32)
        make_identity(nc, ident128[:, :])
        iotaF = csb[0:1, 0:16]
        rowF = csb[0:1, 16:32]
        colF = csb[0:1, 32:48]
        thresh3 = csb[0:1, 48:51]
        c99 = csb[0:1, 56:72]
        ident8 = csb[0:8, 80:88]

        bdt = mybir.dt.bfloat16 if variant == "bf16" else F32

        for ib in range(NB):
            # ---------------- load + interleave + Gram matmuls ----------
            psum = psum_pool.tile([128, 128], F32)
            for tb in range(NTB):
                a_tb = a_pool.tile([128, NV * D], F32)
                av = a_tb[:, :].rearrange("p (v d) -> p v d", v=NV)
                # preds: [S, T, NB, D] -> rows t of this t-block, all 4 sp
                nc.sync.dma_start(
                    out=av[:, 0:S, :],
                    in_=preds_t[:, tb * 128 : (tb + 1) * 128, ib, :].transpose(
                        [1, 0, 2]
                    ),
                )
                # gts: [S, NB, T, D]
                nc.sync.dma_start(
                    out=av[:, S : 2 * S, :],
                    in_=gts_t[:, ib, tb * 128 : (tb + 1) * 128, :].transpose(
                        [1, 0, 2]
                    ),
                )

                # interleave (v, d) -> (g, i, v) with d = 16 g + i, so each
                # matmul operand is one contiguous [128, 128] slab whose
                # column order (i outer, v inner) puts the per-d Gram
                # contributions on 8x8 diagonal blocks.  The copy also
                # casts fp32 -> bf16 (variant bf16).
                b_tb = b_pool.tile([128, NV * D], bdt)
                src = av.transpose([0, 2, 1]).rearrange(
                    "p (g i) v -> p g i v", i=16
                )
                dstv = b_tb[:, :].rearrange("p (g i v) -> p g i v", i=16, v=NV)
                eng = nc.vector if tb % 2 == 0 else nc.scalar
                if eng is nc.vector:
                    eng.tensor_copy(out=dstv, in_=src)
                else:
                    eng.copy(out=dstv, in_=src)

                for g in range(D // 16):
                    op = b_tb[:, g * 128 : (g + 1) * 128]
                    first = tb == 0 and g == 0
                    last = tb == NTB - 1 and g == D // 16 - 1
                    nc.tensor.matmul(
                        psum[:, :], lhsT=op, rhs=op, start=first, stop=last
                    )

            # ---------------- diagonal-block reduction (on PE) ------------
            # Engine APs must start at 32-aligned partitions, so VectorE
            # cannot read the 8x8 blocks at partition 8q directly.  Instead
            # use selector matmuls: I128[:,8q:8q+8].T @ C[:,8q:8q+8] lands
            # block q on partitions 0:8, and PSUM accumulation sums over q.
            c_sb = small_pool.tile([128, 128], F32)
            nc.vector.tensor_copy(out=c_sb[:, :], in_=psum[:, :])
            psg = psumf_pool.tile([8, 8], F32, tag="psg")
            for q in range(16):
                nc.tensor.matmul(
                    psg[:, :],
                    lhsT=ident128[:, 8 * q : 8 * q + 8],
                    rhs=c_sb[:, 8 * q : 8 * q + 8],
                    start=(q == 0),
                    stop=(q == 15),
                )
            acc = small_pool.tile([8, 8], F32)
            nc.vector.tensor_copy(out=acc[:, :], in_=psg[:, :])

            # ---------------- flatten Gram to one partition ----------------
            psf = psumf_pool.tile([1, 72], F32)
            for p in range(8):
                nc.tensor.matmul(
                    psf[0:1, 8 * p : 8 * p + 8],
                    lhsT=ident8[:, p : p + 1],
                    rhs=acc[:, :],
                    start=True,
                    stop=True,
                )

            flat = small_pool.tile([1, 72], F32)
            nc.vector.tensor_copy(out=flat[0:1, 0:64], in_=psf[0:1, 0:64])
            nc.sync.dma_start(out=gram_t[ib : ib + 1, :], in_=flat[0:1, 0:64])

            # ---------------- dist = sqrt(relu(pn + gn - 2*cross)) -------
            g9 = flat[0:1, 0:72].rearrange("p (a b) -> p a b", b=9)
            pn = g9[:, 0:4, 0:1].broadcast_to((1, 4, 4))
            gn = g9[:, 4:8, 0:1].transpose([0, 2, 1]).broadcast_to((1, 4, 4))
            cross = flat[0:1, 0:64].rearrange("p (a b) -> p a b", b=8)[:, 0:4, 4:8]

            d2 = small_pool.tile([1, 16], F32)
            d2v = d2[0:1, :].rearrange("p (a b) -> p a b", b=4)
            tmp16 = small_pool.tile([1, 16], F32)
            tmp16v = tmp16[0:1, :].rearrange("p (a b) -> p a b", b=4)
            dd = small_pool.tile([1, 16], F32)

            nc.vector.tensor_add(out=d2v, in0=pn, in1=gn)
            nc.vector.tensor_scalar(
                out=tmp16v, in0=cross, scalar1=-2.0, scalar2=None, op0=mybir.AluOpType.mult
            )
            nc.vector.tensor_add(out=d2[:, :], in0=d2[:, :], in1=tmp16[:, :])
            nc.vector.tensor_scalar_max(out=d2[:, :], in0=d2[:, :], scalar1=0.0)
            nc.scalar.sqrt(out=dd[:, :], in_=d2[:, :])

            # ---------------- greedy matching ----------------
            lossb = small_pool.tile([1, 1], F32)
            mval = small_pool.tile([1, 1], F32)
            jf = small_pool.tile([1, 1], F32)
            rf = small_pool.tile([1, 1], F32)
            cf = small_pool.tile([1, 1], F32)
            t3 = small_pool.tile([1, 3], F32)
            mask16 = small_pool.tile([1, 16], mybir.dt.int32)
            jsel = small_pool.tile([1, 16], F32)
            mrowB = small_pool.tile([1, 16], F32)
            mcolB = small_pool.tile([1, 16], F32)

            nc.vector.memset(lossb[:, :], 0.0)
            for it in range(S):
                nc.vector.tensor_reduce(
                    out=mval[:, :],
                    in_=dd[:, :],
                    axis=mybir.AxisListType.X,
                    op=mybir.AluOpType.min,
                )
                nc.vector.tensor_add(
                    out=lossb[:, :], in0=lossb[:, :], in1=mval[:, :]
                )
                if it == S - 1:
                    break
                nc.vector.tensor_scalar(
                    out=mask16[:, :],
                    in0=dd[:, :],
                    scalar1=mval[:, 0:1],
                    scalar2=None,
                    op0=mybir.AluOpType.is_le,
                )
                nc.vector.select(
                    out=jsel[:, :], mask=mask16[:, :], on_true=iotaF, on_false=c99
                )
                nc.vector.tensor_reduce(
                    out=jf[:, :],
                    in_=jsel[:, :],
                    axis=mybir.AxisListType.X,
                    op=mybir.AluOpType.min,
                )
                # rf = floor(jf/4) via count of thresholds <= jf
                nc.vector.tensor_scalar(
                    out=t3[:, :],
                    in0=thresh3,
                    scalar1=jf[:, 0:1],
                    scalar2=None,
                    op0=mybir.AluOpType.is_le,
                )
                nc.vector.tensor_reduce(
                    out=rf[:, :],
                    in_=t3[:, :],
                    axis=mybir.AxisListType.X,
                    op=mybir.AluOpType.add,
                )
                # cf = jf - 4*rf
                nc.vector.tensor_scalar(
                    out=cf[:, :],
                    in0=rf[:, :],
                    scalar1=-4.0,
                    scalar2=jf[:, 0:1],
                    op0=mybir.AluOpType.mult,
                    op1=mybir.AluOpType.add,
                )
                nc.vector.tensor_scalar(
                    out=mrowB[:, :],
                    in0=rowF,
                    scalar1=rf[:, 0:1],
                    scalar2=BIG,
                    op0=mybir.AluOpType.is_equal,
                    op1=mybir.AluOpType.mult,
                )
                nc.vector.tensor_scalar(
                    out=mcolB[:, :],
                    in0=colF,
                    scalar1=cf[:, 0:1],
                    scalar2=BIG,
                    op0=mybir.AluOpType.is_equal,
                    op1=mybir.AluOpType.mult,
                )
                nc.vector.tensor_add(
                    out=dd[:, :], in0=dd[:, :], in1=mrowB[:, :]
                )
                nc.vector.tensor_add(
                    out=dd[:, :], in0=dd[:, :], in1=mcolB[:, :]
                )

            nc.sync.dma_start(out=loss_t[ib : ib + 1, :], in_=lossb[:, :])


_NC_CACHE: dict = {}


def _get_nc(variant: str = VARIANT):
    if variant not in _NC_CACHE:
        _NC_CACHE[variant] = build_nc(variant)
    return _NC_CACHE[variant]


def shard_inputs(preds: np.ndarray, gts: np.ndarray):
    in_maps = []
    for c in range(N_CORES):
        b0 = c * NB
        in_maps.append(
            {
                "preds": np.ascontiguousarray(preds[:, :, b0 : b0 + NB, :]),
                "gts": np.ascontiguousarray(gts[:, b0 : b0 + NB, :, :]),
                "consts": CONSTS,
            }
        )
    return in_maps


kernel_last_results = None


def kernel(preds: np.ndarray, gts: np.ndarray) -> np.ndarray:
    global kernel_last_results
    nc = _get_nc()
    in_maps = shard_inputs(np.asarray(preds), np.asarray(gts))
    trace = os.environ.get("MINLOSS_TRACE", "1") == "1"
    res = run_bass_kernel_spmd(
        nc, in_maps, core_ids=list(range(N_CORES)), trace=trace
    )
    kernel_last_results = res
    total = 0.0
    for c in range(N_CORES):
        total += float(res.results[c]["loss"].sum())
    return np.array(total, dtype=np.float32)


# revision 16
# speedup vs baseline: 1.3203x; 1.3203x over previous
"""Trainium2 Bass kernel for nn_MinLoss_12343736009330.

Math: the reference loss is
    loss = sum_{b,s} || pf[b,s] - gf[b,match[b,s]] ||_2
where pf/gf are the per-(batch, source) flattened [L=T*D] signals, and match
is a greedy assignment on the 4x4 Euclidean cdist.  Since
    ||pf[s] - gf[m]||^2 = pn[s] + gn[m] - 2 <pf[s], gf[m]>,
the whole computation reduces to the per-batch 8x8 Gram matrix of the
8 vectors {pf[0..4], gf[0..4]} plus a tiny 4x4 greedy matching.

Sharding: batch axis (16) across 8 cores -> 2 batches/core.  Each core
computes its 2 Gram matrices on the TensorEngine (contraction over t in
128-row tiles; operand columns interleaved (d, v) so that the 8x8 diagonal
blocks of each accumulated matmul hold per-d-slice Gram contributions),
extracts + reduces the diagonal blocks, computes dist = sqrt(pn+gn-2G),
runs the greedy matching on-device with VectorE ops, and writes one loss
scalar per batch.  Host sums the 16 partial scalars.
"""

import os
import sys

import numpy as np

try:
    import concourse.bass as bass  # noqa: F401
except ImportError:
    sys.path.insert(0, "/opt/trn_rl_repo")

import concourse.bass as bass
import concourse.tile as tile
from concourse import bacc, mybir
from concourse.bass_utils import run_bass_kernel_spmd
from concourse.masks import make_identity


def _install_ntff_hook_shim():
    """The bare agent image lacks ``antenv.axon_hooks``, so trace=True under
    axon would ImportError.  Recreate the module with the ctypes-based NTFF
    hook from trn_agent_boot (degrades to hook=None if unavailable)."""
    import types

    try:
        import antenv.axon_hooks  # noqa: F401

        return
    except ImportError:
        pass
    hook = None
    try:
        from trn_agent_boot.trn_boot import _ntff_profile_via_ctypes

        so_path = "/opt/axon/libaxon_pjrt.so"
        if os.path.exists(so_path):
            hook = _ntff_profile_via_ctypes(so_path)
    except Exception:
        hook = None
    import antenv

    mod = types.ModuleType("antenv.axon_hooks")
    mod.get_axon_ntff_profile_hook = lambda: hook  # type: ignore[attr-defined]

    def _set(h):
        nonlocal hook
        hook = h

    mod.set_axon_ntff_profile_hook = _set  # type: ignore[attr-defined]
    sys.modules["antenv.axon_hooks"] = mod
    antenv.axon_hooks = mod


_install_ntff_hook_shim()

F32 = mybir.dt.float32

S, T, B, D = 4, 512, 16, 512
N_CORES = 8
NB = B // N_CORES          # batches per core
NTB = T // 128             # t-blocks per batch
NV = 2 * S                 # 8 vectors per batch (4 preds + 4 gts)

# "bf16": interleave-copy casts to bf16; matmuls at 1 cyc/col (fp32 PSUM).
# "fp32": interleave-copy stays fp32; matmuls at 4 cyc/col (exact).
VARIANT = os.environ.get("MINLOSS_VARIANT", "bf16")
BIG = 1.0e30


def _build_consts() -> np.ndarray:
    """Host-side constant block, DMA'd once: [8, 96] fp32.

    row 0: cols 0:16 iota(16), 16:32 row-id (j//4), 32:48 col-id (j%4),
           48:51 thresholds {4,8,12}, 56:72 the value 99.0.
    rows 0..8, cols 80:88: 8x8 identity (for the flatten matmuls).
    """
    c = np.zeros((8, 96), np.float32)
    j = np.arange(16)
    c[0, 0:16] = j
    c[0, 16:32] = j // 4
    c[0, 32:48] = j % 4
    c[0, 48:51] = [4.0, 8.0, 12.0]
    c[0, 56:72] = 99.0
    c[0:8, 80:88] = np.eye(8, dtype=np.float32)
    return c


CONSTS = _build_consts()


def build_nc(variant: str = VARIANT):
    nc = bacc.Bacc(
        "TRN2",
        target_bir_lowering=False,
        debug=False,
        enable_asserts=True,
        num_devices=N_CORES,
    )
    # xa: host-side pre-interleaved shard.  xa[b, tb, p, g*128 + i*8 + v]
    # holds vector v's value at t = 128*tb + p, d = 16*g + i (v 0..3 preds,
    # 4..7 gts).  Each matmul operand is then one contiguous [128,128] slab.
    xa_t = nc.dram_tensor(
        "xa", [NB, NTB, 128, NV * D], F32, kind="ExternalInput"
    ).ap()
    consts_t = nc.dram_tensor("consts", [8, 96], F32, kind="ExternalInput").ap()
    loss_t = nc.dram_tensor("loss", [NB, 1], F32, kind="ExternalOutput").ap()
    gram_t = nc.dram_tensor("gram", [NB, 64], F32, kind="ExternalOutput").ap()

    with tile.TileContext(nc) as tc:
        _build_tile(tc, xa_t, consts_t, loss_t, gram_t, variant)

    nc.compile()
    return nc


def _build_tile(tc, xa_t, consts_t, loss_t, gram_t, variant):
    nc = tc.nc
    import contextlib

    ctx = contextlib.ExitStack()
    with ctx:
        b_pool = ctx.enter_context(tc.tile_pool(name="b", bufs=2 * NTB))
        psum_pool = ctx.enter_context(tc.tile_pool(name="psum", bufs=2, space="PSUM"))
        psumf_pool = ctx.enter_context(tc.tile_pool(name="psumf", bufs=2, space="PSUM"))
        consts_pool = ctx.enter_context(tc.tile_pool(name="consts", bufs=1))
        small_pool = ctx.enter_context(tc.tile_pool(name="small", bufs=2))

        csb = consts_pool.tile([8, 96], F32)
        nc.sync.dma_start(out=csb[:, :], in_=consts_t[:, :])
        ident128 = consts_pool.tile([128, 128], F32)
        make_identity(nc, ident128[:, :])
        iotaF = csb[0:1, 0:16]
        rowF = csb[0:1, 16:32]
        colF = csb[0:1, 32:48]
        thresh3 = csb[0:1, 48:51]
        c99 = csb[0:1, 56:72]
        ident8 = csb[0:8, 80:88]

        bdt = mybir.dt.bfloat16 if variant == "bf16" else F32

        for ib in range(NB):
            # ---------------- load (casting DMA) + Gram matmuls ----------
            psum = psum_pool.tile([128, 128], F32)
            for tb in range(NTB):
                b_tb = b_pool.tile([128, NV * D], bdt)
                if variant == "bf16":
                    # SWDGE DMA with inline fp32 -> bf16 cast
                    nc.gpsimd.dma_start(out=b_tb[:, :], in_=xa_t[ib, tb, :, :])
                else:
                    nc.sync.dma_start(out=b_tb[:, :], in_=xa_t[ib, tb, :, :])

                for g in range(D // 16):
                    op = b_tb[:, g * 128 : (g + 1) * 128]
                    first = tb == 0 and g == 0
                    last = tb == NTB - 1 and g == D // 16 - 1
                    nc.tensor.matmul(
                        psum[:, :], lhsT=op, rhs=op, start=first, stop=last
                    )

            # ---------------- diagonal-block reduction (on PE) ------------
            # Engine APs must start at 32-aligned partitions, so VectorE
            # cannot read the 8x8 blocks at partition 8q directly.  Instead
            # use selector matmuls: I128[:,8q:8q+8].T @ C[:,8q:8q+8] lands
            # block q on partitions 0:8, and PSUM accumulation sums over q.
            c_sb = small_pool.tile([128, 128], F32)
            nc.vector.tensor_copy(out=c_sb[:, :], in_=psum[:, :])
            psg = psumf_pool.tile([8, 8], F32, tag="psg")
            for q in range(16):
                nc.tensor.matmul(
                    psg[:, :],
                    lhsT=ident128[:, 8 * q : 8 * q + 8],
                    rhs=c_sb[:, 8 * q : 8 * q + 8],
                    start=(q == 0),
                    stop=(q == 15),
                )
            acc = small_pool.tile([8, 8], F32)
            nc.vector.tensor_copy(out=acc[:, :], in_=psg[:, :])

            # ---------------- flatten Gram to one partition ----------------
            psf = psumf_pool.tile([1, 72], F32)
            for p in range(8):
                nc.tensor.matmul(
                    psf[0:1, 8 * p : 8 * p + 8],
                    lhsT=ident8[:, p : p + 1],
                    rhs=acc[:, :],
                    start=True,
                    stop=True,
                )

            flat = small_pool.tile([1, 72], F32)
            nc.vector.tensor_copy(out=flat[0:1, 0:64], in_=psf[0:1, 0:64])
            nc.sync.dma_start(out=gram_t[ib : ib + 1, :], in_=flat[0:1, 0:64])

            # ---------------- dist = sqrt(relu(pn + gn - 2*cross)) -------
            g9 = flat[0:1, 0:72].rearrange("p (a b) -> p a b", b=9)
            pn = g9[:, 0:4, 0:1].broadcast_to((1, 4, 4))
            gn = g9[:, 4:8, 0:1].transpose([0, 2, 1]).broadcast_to((1, 4, 4))
            cross = flat[0:1, 0:64].rearrange("p (a b) -> p a b", b=8)[:, 0:4, 4:8]

            d2 = small_pool.tile([1, 16], F32)
            d2v = d2[0:1, :].rearrange("p (a b) -> p a b", b=4)
            tmp16 = small_pool.tile([1, 16], F32)
            tmp16v = tmp16[0:1, :].rearrange("p (a b) -> p a b", b=4)
            dd = small_pool.tile([1, 16], F32)

            nc.vector.tensor_add(out=d2v, in0=pn, in1=gn)
            nc.vector.tensor_scalar(
                out=tmp16v, in0=cross, scalar1=-2.0, scalar2=None, op0=mybir.AluOpType.mult
            )
            nc.vector.tensor_add(out=d2[:, :], in0=d2[:, :], in1=tmp16[:, :])
            nc.vector.tensor_scalar_max(out=d2[:, :], in0=d2[:, :], scalar1=0.0)
            nc.scalar.sqrt(out=dd[:, :], in_=d2[:, :])

            # ---------------- greedy matching ----------------
            lossb = small_pool.tile([1, 1], F32)
            mval = small_pool.tile([1, 1], F32)
            jf = small_pool.tile([1, 1], F32)
            rf = small_pool.tile([1, 1], F32)
            cf = small_pool.tile([1, 1], F32)
            t3 = small_pool.tile([1, 3], F32)
            mask16 = small_pool.tile([1, 16], mybir.dt.int32)
            jsel = small_pool.tile([1, 16], F32)
            mrowB = small_pool.tile([1, 16], F32)
            mcolB = small_pool.tile([1, 16], F32)

            nc.vector.memset(lossb[:, :], 0.0)
            for it in range(S):
                nc.vector.tensor_reduce(
                    out=mval[:, :],
                    in_=dd[:, :],
                    axis=mybir.AxisListType.X,
                    op=mybir.AluOpType.min,
                )
                nc.vector.tensor_add(
                    out=lossb[:, :], in0=lossb[:, :], in1=mval[:, :]
                )
                if it == S - 1:
                    break
                nc.vector.tensor_scalar(
                    out=mask16[:, :],
                    in0=dd[:, :],
                    scalar1=mval[:, 0:1],
                    scalar2=None,
                    op0=mybir.AluOpType.is_le,
                )
                nc.vector.select(
                    out=jsel[:, :], mask=mask16[:, :], on_true=iotaF, on_false=c99
                )
                nc.vector.tensor_reduce(
                    out=jf[:, :],
                    in_=jsel[:, :],
                    axis=mybir.AxisListType.X,
                    op=mybir.AluOpType.min,
                )
                # rf = floor(jf/4) via count of thresholds <= jf
                nc.vector.tensor_scalar(
                    out=t3[:, :],
                    in0=thresh3,
                    scalar1=jf[:, 0:1],
                    scalar2=None,
                    op0=mybir.AluOpType.is_le,
                )
                nc.vector.tensor_reduce(
                    out=rf[:, :],
                    in_=t3[:, :],
                    axis=mybir.AxisListType.X,
                    op=mybir.AluOpType.add,
                )
                # cf = jf - 4*rf
                nc.vector.tensor_scalar(
                    out=cf[:, :],
                    in0=rf[:, :],
                    scalar1=-4.0,
                    scalar2=jf[:, 0:1],
                    op0=mybir.AluOpType.mult,
                    op1=mybir.AluOpType.add,
                )
                nc.vector.tensor_scalar(
                    out=mrowB[:, :],
                    in0=rowF,
                    scalar1=rf[:, 0:1],
                    scalar2=BIG,
                    op0=mybir.AluOpType.is_equal,
                    op1=mybir.AluOpType.mult,
                )
                nc.vector.tensor_scalar(
                    out=mcolB[:, :],
                    in0=colF,
                    scalar1=cf[:, 0:1],
                    scalar2=BIG,
                    op0=mybir.AluOpType.is_equal,
                    op1=mybir.AluOpType.mult,
                )
                nc.vector.tensor_add(
                    out=dd[:, :], in0=dd[:, :], in1=mrowB[:, :]
                )
                nc.vector.tensor_add(
                    out=dd[:, :], in0=dd[:, :], in1=mcolB[:, :]
                )

            nc.sync.dma_start(out=loss_t[ib : ib + 1, :], in_=lossb[:, :])


_NC_CACHE: dict = {}


def _get_nc(variant: str = VARIANT):
    if variant not in _NC_CACHE:
        _NC_CACHE[variant] = build_nc(variant)
    return _NC_CACHE[variant]


def shard_inputs(preds: np.ndarray, gts: np.ndarray):
    """Build the interleaved layout X[b, tb, p, g*128 + i*8 + v] and slice
    per core (b is outermost, so per-core slices are contiguous views)."""
    X = np.empty((B, NTB, 128, 32, 16, NV), np.float32)
    # preds [S, T, B, D] -> [b, tb, p, g, i, s]
    X[..., 0:S] = preds.reshape(S, NTB, 128, B, 32, 16).transpose(3, 1, 2, 4, 5, 0)
    # gts [S, B, T, D] -> [b, tb, p, g, i, s]
    X[..., S : 2 * S] = gts.reshape(S, B, NTB, 128, 32, 16).transpose(
        1, 2, 3, 4, 5, 0
    )
    X = X.reshape(B, NTB, 128, NV * D)
    in_maps = []
    for c in range(N_CORES):
        b0 = c * NB
        in_maps.append({"xa": X[b0 : b0 + NB], "consts": CONSTS})
    return in_maps


kernel_last_results = None


def kernel(preds: np.ndarray, gts: np.ndarray) -> np.ndarray:
    global kernel_last_results
    nc = _get_nc()
    in_maps = shard_inputs(np.asarray(preds), np.asarray(gts))
    trace = os.environ.get("MINLOSS_TRACE", "1") == "1"
    res = run_bass_kernel_spmd(
        nc, in_maps, core_ids=list(range(N_CORES)), trace=trace
    )
    kernel_last_results = res
    total = 0.0
    for c in range(N_CORES):
        total += float(res.results[c]["loss"].sum())
    return np.array(total, dtype=np.float32)
